# revision 6
# baseline (speedup 1.0000x reference)
"""Trainium2 Bass kernel for nn_KnowledgeFusion.

Math (b=8, H=W=32, d=o=256, n_obj=15):
  embs_aug = concat([embs, mean(embs)])                  [b,16,256]
  mask     = rasterized boxes (rounded to PATCH_SIZE=2)  [b,16,1024] in {0,1}
  proj     = patches @ Wp                                [b,1024,256]
  inj      = embs_aug @ We                               [b,16,256]
  s[hw]    = sum_n mask[n,hw]   (>=1: image box row)
  out      = proj + (mask^T @ inj) / s[:,None]           [b,1024,256]

The mean-emb row folds away: with inj_k = embs_k @ We (k<15),
  outT[o,hw] = Wp^T @ patchesT + inj^T @ ((mask + 1/15) * recB)
where recB = 1/s, s integer in 1..16, recovered exactly via a one-hot
(is_equal against a per-partition constant) collapsed by a tiny matmul
against weights 1/(p+1).

v2 layout: the pixel halves h=0/h=1 live on partition strips 0:16 and
32:48.  Because DVE cost scales with free-size only (not partitions)
and matmul cost with streamed columns only, the whole mask -> s ->
one-hot -> recB -> maskN chain is FUSED across both halves: one DVE op
each for mask/is_eq/maskN on [48,512], and one 512-column matmul each
for s and recB using [48,64] block-diagonal weights (rows 16:31 flow
zeros).  The strip split is kept so the four bank-closing inj matmuls
(weights on rows 0:15 vs 32:47) still pair up concurrently on the PE.

PSUM banks close in pairs as early as the data allows; each pair
evacuates on ACT+DVE concurrently and its output DMA is triggered
immediately from the same engines, overlapping the remaining PE work.
Inputs are spread over four HWDGE queues ordered by first use, with
the We/embs half of the weights blob landing before Wp.
"""

import sys

sys.path.insert(0, "/opt/trn_rl_repo")

import numpy as np

import concourse.bass as bass
import concourse.bacc as bacc
import concourse.mybir as mybir
from concourse import tile
from concourse import bass_utils
from concourse.alu_op_type import AluOpType

B, H, W, D = 8, 32, 32, 256
NOBJ, N = 15, 16
HW = H * W
O = 256
P2 = 32  # partition offset of the h=1 group
NP = 48
FP = mybir.dt.float32
BF = mybir.dt.bfloat16
I32 = mybir.dt.int32
AF = mybir.ActivationFunctionType

# weights blob columns (bf16): We0 We1 eTW0 eTW1 Wp0 Wp1
WB_A = 2 * O + 2 * 64  # 640: We + duplicated-embs blocks (first DMA)
WB = WB_A + 2 * O  # 1152 total (second DMA carries Wp)


def _ap(ap, free_dims):
    """AP with explicit free-dim [step, count] pairs (step 0 = broadcast)."""
    return bass.AP(ap.tensor, ap.offset, ap.ap[:1] + free_dims)


def build_nc(debug: bool = False):
    nc = bacc.Bacc("TRN2", target_bir_lowering=False, debug=debug, num_devices=B)

    loc = nc.dram_tensor("loc", [NP, 4], I32, kind="ExternalInput")
    wb = nc.dram_tensor("wb", [128, WB], BF, kind="ExternalInput")
    pT = nc.dram_tensor("pT", [128, 2 * HW], BF, kind="ExternalInput")
    outT = nc.dram_tensor("outT", [128, 2 * HW], BF, kind="ExternalOutput")

    with tile.TileContext(nc) as tc:
        with (
            nc.allow_low_precision(reason="bf16 matmuls, fp32 PSUM accumulation"),
            tc.tile_pool(name="big", bufs=1) as big,
            tc.tile_pool(name="small", bufs=1) as small,
            tc.tile_pool(name="outp", bufs=1) as outp,
            tc.tile_pool(name="psT", bufs=1, space=bass.MemorySpace.PSUM) as psT,
            tc.tile_pool(name="psS", bufs=1, space=bass.MemorySpace.PSUM) as psS,
            tc.tile_pool(name="psI", bufs=1, space=bass.MemorySpace.PSUM) as psI,
        ):
            # ---- input DMAs on the two HWDGE queues, ordered by first use
            # (each dma_start's descriptors spread across all DMA engines):
            #   sync:   loc (heads the mask chain), pT k0h0, k1h0, k1h1
            #   scalar: wb_a (We+eTW -> inj_pre), wb_b (Wp), pT k0h1
            loc_sb = small.tile([NP, 4], I32)
            nc.sync.dma_start(loc_sb[:], loc[:])
            wb_sb = big.tile([128, WB], BF)
            nc.scalar.dma_start(wb_sb[:, 0:WB_A], wb[:, 0:WB_A])
            pT_sb = big.tile([128, 2 * HW], BF)
            nc.sync.dma_start(pT_sb[:, 0:512], pT[:, 0:512])  # k0 h0
            nc.scalar.dma_start(wb_sb[:, WB_A:WB], wb[:, WB_A:WB])
            nc.sync.dma_start(pT_sb[:, 1024:1536], pT[:, 1024:1536])  # k1 h0
            nc.scalar.dma_start(pT_sb[:, 512:1024], pT[:, 512:1024])  # k0 h1
            nc.sync.dma_start(pT_sb[:, 1536:2048], pT[:, 1536:2048])  # k1 h1

            We_sb = [wb_sb[:, O * k : O * (k + 1)] for k in range(2)]
            eTW_sb = [
                wb_sb[:, 2 * O + 64 * k : 2 * O + 64 * (k + 1)] for k in range(2)
            ]
            Wp_sb = [wb_sb[:, WB_A + O * k : WB_A + O * (k + 1)] for k in range(2)]

            # ---- constants (all off the critical path)
            # W_s: block "diagonal" ones so one matmul sums both halves:
            #   out rows 0:16 = s(h0), rows 32:48 = s(h1), rows 16:32 = 0
            W_s = small.tile([NP, 64], BF, name="W_s")
            nc.gpsimd.memset(W_s[:], 0.0)
            nc.gpsimd.memset(W_s[0:N, 0:N], 1.0)
            nc.gpsimd.memset(W_s[P2 : P2 + N, P2 : P2 + N], 1.0)

            grid_i = small.tile([NP, 32], I32, name="grid")
            nc.gpsimd.iota(grid_i[:], pattern=[[1, 32]], base=0, channel_multiplier=0)
            grid_f = small.tile([NP, 32], FP, name="gridf")
            nc.vector.tensor_copy(grid_f[:], grid_i[:])
            grid2_f = small.tile([NP, 32], FP, name="grid2f")
            nc.vector.tensor_scalar(
                grid2_f[:], grid_f[:], 2.0, None, op0=AluOpType.subtract
            )
            # kvec[p] = (p & 31) + 1  in fp32 (1..16 in both groups)
            kidx = small.tile([NP, 1], I32, name="kidx")
            nc.gpsimd.iota(kidx[:], pattern=[[1, 1]], base=0, channel_multiplier=1)
            kid1 = small.tile([NP, 1], I32, name="kid1")
            nc.vector.tensor_scalar(
                kid1[:], kidx[:], 31, None, op0=AluOpType.bitwise_and
            )
            kid2 = small.tile([NP, 1], I32, name="kid2")
            nc.vector.tensor_scalar(kid2[:], kid1[:], 1, None, op0=AluOpType.add)
            kvec = small.tile([NP, 1], FP, name="kvec")
            nc.vector.tensor_copy(kvec[:], kid2[:])
            wn = small.tile([NP, 1], FP, name="wn")
            nc.vector.reciprocal(wn[:], kvec[:])
            # W_r: block weights 1/(p+1) collapsing the one-hot to 1/s for
            # both halves at once; out rows 0:16 = recB(h0), 32:48 = recB(h1)
            W_r = small.tile([NP, 64], BF, name="W_r")
            nc.gpsimd.memset(W_r[:], 0.0)
            nc.vector.tensor_copy(W_r[0:N, 0:N], _ap(wn[0:N, 0:1], [[0, N]]))
            nc.vector.tensor_copy(
                W_r[P2 : P2 + N, P2 : P2 + N], _ap(wn[P2 : P2 + N, 0:1], [[0, N]])
            )
            # y-grid per strip: rows 0:16 cover y=0..15 (h0), rows 32:48
            # y=16..31 (h1); rows 16:32 duplicate h0 (harmless junk)
            yoff_i = small.tile([NP, 1], I32, name="yoff_i")
            nc.vector.tensor_scalar(
                yoff_i[:], kidx[:], 32, None, op0=AluOpType.bitwise_and
            )
            yoff2_i = small.tile([NP, 1], I32, name="yoff2_i")
            nc.vector.tensor_scalar(
                yoff2_i[:], yoff_i[:], 1, None, op0=AluOpType.logical_shift_right
            )
            yoff_f = small.tile([NP, 1], FP, name="yoff_f")
            nc.vector.tensor_copy(yoff_f[:], yoff2_i[:])
            ygrid_i = small.tile([NP, N], I32, name="ygrid_i")
            nc.gpsimd.iota(ygrid_i[:], pattern=[[1, N]], base=0, channel_multiplier=0)
            ygrid_f0 = small.tile([NP, N], FP, name="ygrid_f0")
            nc.vector.tensor_copy(ygrid_f0[:], ygrid_i[:])
            ygrid_f = small.tile([NP, N], FP, name="ygrid_f")
            nc.vector.tensor_scalar(
                ygrid_f[:], ygrid_f0[:], yoff_f[:, 0:1], None, op0=AluOpType.add
            )
            ygrid2_f = small.tile([NP, N], FP, name="ygrid2_f")
            nc.vector.tensor_scalar(
                ygrid2_f[:], ygrid_f[:], 2.0, None, op0=AluOpType.subtract
            )

            # ---- boxes: round starts down; ends handled via shifted grid
            boxes_i = small.tile([NP, 4], I32, name="boxes_i")
            nc.vector.tensor_scalar(
                boxes_i[:], loc_sb[:], -2, None, op0=AluOpType.bitwise_and
            )
            boxes = small.tile([NP, 4], FP, name="boxes")
            nc.vector.tensor_copy(boxes[:], boxes_i[:])

            # ---- row/col interval masks (bf16 0/1); rowm2 is per-strip 16-col
            colm = small.tile([NP, 32], BF, name="colm")
            rowm2 = small.tile([NP, N], BF, name="rowm2")
            tmp_x = small.tile([NP, 32], FP, name="tmp_x")
            tmp_y = small.tile([NP, N], FP, name="tmp_y")
            # grid-2 < (end&-2)  ==  grid < (end&-2)+2
            nc.vector.tensor_scalar(
                tmp_x[:], grid2_f[:], boxes[:, 3:4], None, op0=AluOpType.is_lt
            )
            nc.vector.scalar_tensor_tensor(
                colm[:], grid_f[:], boxes[:, 1:2], tmp_x[:],
                op0=AluOpType.is_ge, op1=AluOpType.mult,
            )
            nc.vector.tensor_scalar(
                tmp_y[:], ygrid2_f[:], boxes[:, 2:3], None, op0=AluOpType.is_lt
            )
            nc.vector.scalar_tensor_tensor(
                rowm2[:], ygrid_f[:], boxes[:, 0:1], tmp_y[:],
                op0=AluOpType.is_ge, op1=AluOpType.mult,
            )

            # ---- fused chain tiles: h=0 rows 0:16, h=1 rows 32:48
            mask = small.tile([NP, 512], BF, name="mask")
            ind = small.tile([NP, 512], BF, name="ind")
            maskN = small.tile([NP, 512], BF, name="maskN")
            psumS = psS.tile([64, 512], FP, name="psS")
            psumR = psS.tile([64, 512], FP, name="psR")
            psumI = psI.tile([64, O], FP, name="psI")
            psum = [[psT.tile([128, 512], FP, name=f"ps{h}{oc}") for oc in range(2)]
                    for h in range(2)]

            # ---- single fused DVE ops over all 48 partitions
            def mask_op():
                nc.vector.tensor_tensor(
                    _ap(mask[:, 0:512], [[W, N], [1, W]]),
                    _ap(rowm2[:, 0:N], [[1, N], [0, W]]),
                    _ap(colm[:, :], [[0, N], [1, W]]),
                    op=AluOpType.mult,
                )

            def s_mm():
                return nc.tensor.matmul(
                    psumS[:], W_s[:], mask[:], start=True, stop=True
                )

            def iseq_op():  # partition p: ind = (s == (p&31)+1); junk rows -> 0
                nc.vector.tensor_scalar(
                    ind[:], psumS[0:NP, :], kvec[:, 0:1], None,
                    op0=AluOpType.is_equal,
                )

            def ind_mm():
                return nc.tensor.matmul(
                    psumR[:], W_r[:], ind[:], start=True, stop=True
                )

            def maskN_op():
                # (mask + 1/15) * recB  -- the +1/15 carries the mean-emb row
                nc.vector.scalar_tensor_tensor(
                    maskN[:], mask[:], 1.0 / NOBJ, psumR[0:NP, :],
                    op0=AluOpType.add, op1=AluOpType.mult,
                )

            def inj_pre_mm(k):
                return nc.tensor.matmul(
                    psumI[:], eTW_sb[k][:], We_sb[k][:],
                    start=(k == 0), stop=(k == 1),
                )

            def proj_mm(h, oc, k):
                return nc.tensor.matmul(
                    psum[h][oc][:],
                    Wp_sb[k][:, 128 * oc : 128 * (oc + 1)],
                    pT_sb[:, HW * k + 512 * h : HW * k + 512 * (h + 1)],
                    start=(k == 0), stop=False,
                )

            def inj_mm(h, oc):
                p0 = P2 * h
                return nc.tensor.matmul(
                    psum[h][oc][:],
                    inj_sb[p0 : p0 + NOBJ, 128 * oc : 128 * (oc + 1)],
                    maskN[p0 : p0 + NOBJ, :],
                    start=False, stop=True,
                )

            # ---- emission order doubles as per-engine FIFO order and
            # MUST be topological (Tile tracks deps by trace order).
            mask_op()

            pe = []
            pe.append(inj_pre_mm(0))
            pe.append(inj_pre_mm(1))
            inj_sb = small.tile([NP, O], BF, name="inj")
            nc.scalar.activation(inj_sb[:], psumI[0:NP, :], AF.Copy)

            pe.append(proj_mm(0, 0, 0))
            pe.append(s_mm())
            iseq_op()
            pe.append(proj_mm(0, 0, 1))
            pe.append(ind_mm())
            maskN_op()
            pe.append(proj_mm(1, 0, 0))
            pe.append(proj_mm(1, 0, 1))
            pe.append(inj_mm(0, 0))
            pe.append(inj_mm(1, 0))

            # pair 1 evacuates on ACT+DVE and streams out while PE continues
            o_sb = outp.tile([128, 2 * HW], BF, name="osb")
            nc.scalar.activation(o_sb[:, 0:512], psum[0][0][:], AF.Copy)
            nc.vector.tensor_copy(o_sb[:, 512:1024], psum[1][0][:])
            nc.scalar.dma_start(outT[:, 0:512], o_sb[:, 0:512])
            nc.sync.dma_start(outT[:, 512:1024], o_sb[:, 512:1024])

            pe.append(proj_mm(0, 1, 0))
            pe.append(proj_mm(0, 1, 1))
            pe.append(proj_mm(1, 1, 0))
            pe.append(proj_mm(1, 1, 1))
            pe.append(inj_mm(0, 1))
            pe.append(inj_mm(1, 1))
            for a, b in zip(pe, pe[1:]):
                tile.add_dep_helper(b.ins, a.ins, sync=False, reason="PE order")

            nc.scalar.activation(o_sb[:, 1024:1536], psum[0][1][:], AF.Copy)
            nc.vector.tensor_copy(o_sb[:, 1536:2048], psum[1][1][:])
            nc.scalar.dma_start(outT[:, 1024:1536], o_sb[:, 1024:1536])
            nc.sync.dma_start(outT[:, 1536:2048], o_sb[:, 1536:2048])

    nc.compile()
    return nc


def make_in_maps(inputs):
    import ml_dtypes

    bf16 = ml_dtypes.bfloat16
    patches = np.asarray(inputs["patches"], dtype=np.float32)
    embs = np.asarray(inputs["embs"], dtype=np.float32)
    locations = np.asarray(inputs["locations"], dtype=np.int32)
    Wp = np.asarray(inputs["Wp"], dtype=np.float32)
    We = np.asarray(inputs["We"], dtype=np.float32)
    img_box = np.array([[0, 0, H, W]], dtype=np.int32)
    wb_common = np.zeros((128, WB), dtype=np.float32)
    wb_common[:, 0:O] = We[0:128]
    wb_common[:, O : 2 * O] = We[128:256]
    wb_common[:, WB_A : WB_A + O] = Wp[0:128]
    wb_common[:, WB_A + O : WB] = Wp[128:256]
    in_maps = []
    for b in range(B):
        eTb = embs[b].T  # [256, 15]
        wbb = wb_common.copy()
        for k in range(2):
            base = 2 * O + 64 * k
            blk = eTb[128 * k : 128 * (k + 1)]
            wbb[:, base : base + NOBJ] = blk
            wbb[:, base + P2 : base + P2 + NOBJ] = blk
        pTb = patches[b].reshape(HW, D).T  # [256, 1024]
        pT2 = np.concatenate([pTb[0:128], pTb[128:256]], axis=1)  # [128, 2048]
        loc16 = np.concatenate([locations[b], img_box], 0)  # [16, 4]
        in_maps.append(
            {
                "loc": np.ascontiguousarray(np.tile(loc16, (3, 1))),  # [48, 4]
                "wb": np.ascontiguousarray(wbb.astype(bf16)),
                "pT": np.ascontiguousarray(pT2.astype(bf16)),
            }
        )
    return in_maps


_NC = None


def _get_nc():
    global _NC
    if _NC is None:
        _NC = build_nc(debug=False)
    return _NC


def run(inputs, trace: bool = False, **kwargs):
    nc = _get_nc()
    res = bass_utils.run_bass_kernel_spmd(
        nc, make_in_maps(inputs), core_ids=list(range(B)), trace=trace, **kwargs
    )
    outs = []
    for b in range(B):
        arr = np.asarray(res.results[b]["outT"]).astype(np.float32)  # [128, 2048]
        outs.append(np.concatenate([arr[:, 0:HW].T, arr[:, HW : 2 * HW].T], axis=1))
    full = np.stack(outs, axis=0)
    return np.ascontiguousarray(full).astype(np.float32), res


def kernel(**inputs) -> np.ndarray:
    full, _ = run(inputs, trace=False)
    return full


# revision 7
# speedup vs baseline: 1.1302x; 1.1302x over previous
"""Trainium2 Bass kernel for nn_KnowledgeFusion.

Math (b=8, H=W=32, d=o=256, n_obj=15):
  embs_aug = concat([embs, mean(embs)])                  [b,16,256]
  mask     = rasterized boxes (rounded to PATCH_SIZE=2)  [b,16,1024] in {0,1}
  proj     = patches @ Wp                                [b,1024,256]
  inj      = embs_aug @ We                               [b,16,256]
  s[hw]    = sum_n mask[n,hw]   (>=1: image box row)
  out      = proj + (mask^T @ inj) / s[:,None]           [b,1024,256]

The mean-emb row folds away: with inj_k = embs_k @ We (k<15),
  outT[o,hw] = Wp^T @ patchesT + inj^T @ ((mask + 1/15) * (1/s))

v3 layout: the pixel halves h=0/h=1 live on partition strips 0:16 and
32:48.  Because DVE cost scales with free-size only (not partitions)
and matmul cost with streamed columns only, the whole chain is FUSED
across both halves: ONE DVE op each for mask / 1/s / maskN on
[48,512], and ONE 512-column matmul for s using a [48,64]
block-diagonal ones weight (rows 16:31 flow zeros; their s is 0 so
1/s=Inf on junk rows that nothing reads).  The strip split is kept so
the four bank-closing inj matmuls (weights on rows 0:15 vs 32:47)
still pair up concurrently on the PE array's row strips.

All input-independent constants (block-ones W_s, x/y grids) are packed
into the weights blob on the host, so the Vector engine queue starts
directly with the loc-dependent box chain and GpSimd is unused.  The
PE stream is ordered so it never stalls (p-state stays warm): inj_pre,
then proj matmuls with the s matmul slotted second, inj matmuls last.
PSUM banks close in pairs; each pair evacuates on ACT+DVE concurrently
and its output DMA is triggered immediately, overlapping remaining PE
work.  Everything is bf16 (fp32 PSUM accumulation); rel-err ~4.6e-3
against the 2e-2 gate.
"""

import sys

sys.path.insert(0, "/opt/trn_rl_repo")

import numpy as np

import concourse.bass as bass
import concourse.bacc as bacc
import concourse.mybir as mybir
from concourse import tile
from concourse import bass_utils
from concourse.alu_op_type import AluOpType

B, H, W, D = 8, 32, 32, 256
NOBJ, N = 15, 16
HW = H * W
O = 256
P2 = 32  # partition offset of the h=1 group
NP = 48
FP = mybir.dt.float32
BF = mybir.dt.bfloat16
I32 = mybir.dt.int32
AF = mybir.ActivationFunctionType

# weights blob columns (bf16), ordered by first use:
#   We0 We1 eTW0 eTW1 | W_s grid grid2 ygrid ygrid2 | Wp0 Wp1
C_WS = 4 * O + 2 * 64  # after We + duplicated-embs blocks
C_GRID = C_WS + 64
C_GRID2 = C_GRID + 32
C_YG = C_GRID2 + 32
C_YG2 = C_YG + N
WB_A = C_YG2 + N  # 800: first DMA chunk
WB = WB_A + 2 * O  # 1312 total (second DMA carries Wp)


def _ap(ap, free_dims):
    """AP with explicit free-dim [step, count] pairs (step 0 = broadcast)."""
    return bass.AP(ap.tensor, ap.offset, ap.ap[:1] + free_dims)


def build_nc(debug: bool = False):
    nc = bacc.Bacc("TRN2", target_bir_lowering=False, debug=debug, num_devices=B)

    loc = nc.dram_tensor("loc", [NP, 4], I32, kind="ExternalInput")
    wb = nc.dram_tensor("wb", [128, WB], BF, kind="ExternalInput")
    pT = nc.dram_tensor("pT", [128, 2 * HW], BF, kind="ExternalInput")
    outT = nc.dram_tensor("outT", [128, 2 * HW], BF, kind="ExternalOutput")

    with tile.TileContext(nc) as tc:
        with (
            nc.allow_low_precision(reason="bf16 matmuls, fp32 PSUM accumulation"),
            tc.tile_pool(name="big", bufs=1) as big,
            tc.tile_pool(name="small", bufs=1) as small,
            tc.tile_pool(name="outp", bufs=1) as outp,
            tc.tile_pool(name="psT", bufs=1, space=bass.MemorySpace.PSUM) as psT,
            tc.tile_pool(name="psS", bufs=1, space=bass.MemorySpace.PSUM) as psS,
            tc.tile_pool(name="psI", bufs=1, space=bass.MemorySpace.PSUM) as psI,
        ):
            # ---- input DMAs on the two HWDGE queues, ordered by first use:
            #   sync:   loc (heads the mask chain), pT k0, pT k1
            #   scalar: wb_a (We+eTW+consts -> inj_pre/chain), wb_b (Wp)
            loc_sb = small.tile([NP, 4], I32)
            nc.sync.dma_start(loc_sb[:], loc[:])
            wb_sb = big.tile([128, WB], BF)
            nc.scalar.dma_start(wb_sb[:, 0:WB_A], wb[:, 0:WB_A])
            pT_sb = big.tile([128, 2 * HW], BF)
            nc.sync.dma_start(pT_sb[:, 0:HW], pT[:, 0:HW])  # k0
            nc.scalar.dma_start(wb_sb[:, WB_A:WB], wb[:, WB_A:WB])
            nc.sync.dma_start(pT_sb[:, HW : 2 * HW], pT[:, HW : 2 * HW])  # k1

            We_sb = [wb_sb[:, O * k : O * (k + 1)] for k in range(2)]
            eTW_sb = [
                wb_sb[:, 2 * O + 64 * k : 2 * O + 64 * (k + 1)] for k in range(2)
            ]
            W_s = wb_sb[0:NP, C_WS : C_WS + 64]
            grid_b = wb_sb[0:NP, C_GRID : C_GRID + 32]
            grid2_b = wb_sb[0:NP, C_GRID2 : C_GRID2 + 32]
            ygrid_b = wb_sb[0:NP, C_YG : C_YG + N]
            ygrid2_b = wb_sb[0:NP, C_YG2 : C_YG2 + N]
            Wp_sb = [wb_sb[:, WB_A + O * k : WB_A + O * (k + 1)] for k in range(2)]

            # ---- boxes: round starts down; ends handled via shifted grid
            boxes_i = small.tile([NP, 4], I32, name="boxes_i")
            nc.vector.tensor_scalar(
                boxes_i[:], loc_sb[:], -2, None, op0=AluOpType.bitwise_and
            )
            boxes = small.tile([NP, 4], FP, name="boxes")
            nc.vector.tensor_copy(boxes[:], boxes_i[:])

            # ---- row/col interval masks (bf16 0/1); rowm2 is per-strip:
            # rows 0:16 test y=0..15 (h0), rows 32:48 test y=16..31 (h1)
            colm = small.tile([NP, 32], BF, name="colm")
            rowm2 = small.tile([NP, N], BF, name="rowm2")
            tmp_x = small.tile([NP, 32], BF, name="tmp_x")
            tmp_y = small.tile([NP, N], BF, name="tmp_y")
            # grid-2 < (end&-2)  ==  grid < (end&-2)+2
            nc.vector.tensor_scalar(
                tmp_x[:], grid2_b, boxes[:, 3:4], None, op0=AluOpType.is_lt
            )
            nc.vector.scalar_tensor_tensor(
                colm[:], grid_b, boxes[:, 1:2], tmp_x[:],
                op0=AluOpType.is_ge, op1=AluOpType.mult,
            )
            nc.vector.tensor_scalar(
                tmp_y[:], ygrid2_b, boxes[:, 2:3], None, op0=AluOpType.is_lt
            )
            nc.vector.scalar_tensor_tensor(
                rowm2[:], ygrid_b, boxes[:, 0:1], tmp_y[:],
                op0=AluOpType.is_ge, op1=AluOpType.mult,
            )

            # ---- fused chain tiles: h=0 rows 0:16, h=1 rows 32:48
            mask = small.tile([NP, 512], BF, name="mask")
            recB = small.tile([NP, 512], FP, name="recB")
            maskN = small.tile([NP, 512], BF, name="maskN")
            psumS = psS.tile([64, 512], FP, name="psS")
            psumI = psI.tile([64, O], FP, name="psI")
            psum = [[psT.tile([128, 512], FP, name=f"ps{h}{oc}") for oc in range(2)]
                    for h in range(2)]

            def mask_op():  # one fused DVE op; rows 16:32 harmless junk
                nc.vector.tensor_tensor(
                    _ap(mask[:, 0:512], [[W, N], [1, W]]),
                    _ap(rowm2[:, 0:N], [[1, N], [0, W]]),
                    _ap(colm[:, :], [[0, N], [1, W]]),
                    op=AluOpType.mult,
                )

            def s_mm():  # out rows 0:16 = s(h0), 32:48 = s(h1), 16:32 = 0
                return nc.tensor.matmul(
                    psumS[:], W_s, mask[:], start=True, stop=True
                )

            def recB_op():  # 1/s; junk rows hold 1/0=Inf, never read
                nc.vector.reciprocal(recB[:], psumS[0:NP, :])

            def maskN_op():
                # (mask + 1/15) * recB  -- the +1/15 carries the mean-emb row
                nc.vector.scalar_tensor_tensor(
                    maskN[:], mask[:], 1.0 / NOBJ, recB[:],
                    op0=AluOpType.add, op1=AluOpType.mult,
                )

            def inj_pre_mm(k):  # inj on rows 0:15 AND 32:47 via duplicated eT
                return nc.tensor.matmul(
                    psumI[:], eTW_sb[k][:], We_sb[k][:],
                    start=(k == 0), stop=(k == 1),
                )

            def proj_mm(h, oc, k):
                return nc.tensor.matmul(
                    psum[h][oc][:],
                    Wp_sb[k][:, 128 * oc : 128 * (oc + 1)],
                    pT_sb[:, HW * k + 512 * h : HW * k + 512 * (h + 1)],
                    start=(k == 0), stop=False,
                )

            def inj_mm(h, oc):
                p0 = P2 * h
                return nc.tensor.matmul(
                    psum[h][oc][:],
                    inj_sb[p0 : p0 + NOBJ, 128 * oc : 128 * (oc + 1)],
                    maskN[p0 : p0 + NOBJ, :],
                    start=False, stop=True,
                )

            # ---- emission order doubles as per-engine FIFO order and
            # MUST be topological (Tile tracks deps by trace order).
            mask_op()

            pe = []
            pe.append(inj_pre_mm(0))
            pe.append(inj_pre_mm(1))
            inj_sb = small.tile([NP, O], BF, name="inj")
            nc.scalar.activation(inj_sb[:], psumI[0:NP, :], AF.Copy)

            pe.append(proj_mm(0, 0, 0))
            pe.append(s_mm())
            recB_op()
            maskN_op()
            pe.append(proj_mm(0, 0, 1))
            pe.append(proj_mm(1, 0, 0))
            pe.append(proj_mm(1, 0, 1))
            pe.append(proj_mm(0, 1, 0))
            pe.append(proj_mm(0, 1, 1))
            pe.append(proj_mm(1, 1, 0))
            pe.append(proj_mm(1, 1, 1))
            pe.append(inj_mm(0, 0))
            pe.append(inj_mm(1, 0))

            # pair 1 evacuates on ACT+DVE and streams out while PE continues
            o_sb = outp.tile([128, 2 * HW], BF, name="osb")
            nc.scalar.activation(o_sb[:, 0:512], psum[0][0][:], AF.Copy)
            nc.vector.tensor_copy(o_sb[:, 512:1024], psum[1][0][:])
            nc.scalar.dma_start(outT[:, 0:512], o_sb[:, 0:512])
            nc.sync.dma_start(outT[:, 512:1024], o_sb[:, 512:1024])

            pe.append(inj_mm(0, 1))
            pe.append(inj_mm(1, 1))
            for a, b in zip(pe, pe[1:]):
                tile.add_dep_helper(b.ins, a.ins, sync=False, reason="PE order")

            nc.scalar.activation(o_sb[:, 1024:1536], psum[0][1][:], AF.Copy)
            nc.vector.tensor_copy(o_sb[:, 1536:2048], psum[1][1][:])
            nc.scalar.dma_start(outT[:, 1024:1536], o_sb[:, 1024:1536])
            nc.sync.dma_start(outT[:, 1536:2048], o_sb[:, 1536:2048])

    nc.compile()
    return nc


def make_in_maps(inputs):
    import ml_dtypes

    bf16 = ml_dtypes.bfloat16
    patches = np.asarray(inputs["patches"], dtype=np.float32)
    embs = np.asarray(inputs["embs"], dtype=np.float32)
    locations = np.asarray(inputs["locations"], dtype=np.int32)
    Wp = np.asarray(inputs["Wp"], dtype=np.float32)
    We = np.asarray(inputs["We"], dtype=np.float32)
    img_box = np.array([[0, 0, H, W]], dtype=np.int32)

    wb_common = np.zeros((128, WB), dtype=np.float32)
    wb_common[:, 0:O] = We[0:128]
    wb_common[:, O : 2 * O] = We[128:256]
    # W_s: block-diagonal ones summing each strip's 16 masks
    wb_common[0:N, C_WS : C_WS + N] = 1.0
    wb_common[P2 : P2 + N, C_WS + P2 : C_WS + P2 + N] = 1.0
    # x grid (all rows) and strip-offset y grid, plus -2 shifted copies
    wb_common[0:NP, C_GRID : C_GRID + 32] = np.arange(32)[None, :]
    wb_common[0:NP, C_GRID2 : C_GRID2 + 32] = np.arange(32)[None, :] - 2.0
    yg = np.zeros((NP, N), dtype=np.float32)
    yg[:] = np.arange(N)[None, :]
    yg[P2:] += N
    wb_common[0:NP, C_YG : C_YG + N] = yg
    wb_common[0:NP, C_YG2 : C_YG2 + N] = yg - 2.0
    wb_common[:, WB_A : WB_A + O] = Wp[0:128]
    wb_common[:, WB_A + O : WB] = Wp[128:256]

    in_maps = []
    for b in range(B):
        eTb = embs[b].T  # [256, 15]
        wbb = wb_common.copy()
        for k in range(2):
            base = 2 * O + 64 * k
            blk = eTb[128 * k : 128 * (k + 1)]
            wbb[:, base : base + NOBJ] = blk
            wbb[:, base + P2 : base + P2 + NOBJ] = blk
        pTb = patches[b].reshape(HW, D).T  # [256, 1024]
        pT2 = np.concatenate([pTb[0:128], pTb[128:256]], axis=1)  # [128, 2048]
        loc16 = np.concatenate([locations[b], img_box], 0)  # [16, 4]
        in_maps.append(
            {
                "loc": np.ascontiguousarray(np.tile(loc16, (3, 1))),  # [48, 4]
                "wb": np.ascontiguousarray(wbb.astype(bf16)),
                "pT": np.ascontiguousarray(pT2.astype(bf16)),
            }
        )
    return in_maps


_NC = None


def _get_nc():
    global _NC
    if _NC is None:
        _NC = build_nc(debug=False)
    return _NC


def run(inputs, trace: bool = False, **kwargs):
    nc = _get_nc()
    res = bass_utils.run_bass_kernel_spmd(
        nc, make_in_maps(inputs), core_ids=list(range(B)), trace=trace, **kwargs
    )
    outs = []
    for b in range(B):
        arr = np.asarray(res.results[b]["outT"]).astype(np.float32)  # [128, 2048]
        outs.append(np.concatenate([arr[:, 0:HW].T, arr[:, HW : 2 * HW].T], axis=1))
    full = np.stack(outs, axis=0)
    return np.ascontiguousarray(full).astype(np.float32), res


def kernel(**inputs) -> np.ndarray:
    full, _ = run(inputs, trace=False)
    return full
